# revision 23
# baseline (speedup 1.0000x reference)
"""Bahdanau-attention scores kernel for one TRN2 chip (8 NeuronCores).

Reference computation (B=32, S=2048, H=1024):
    energy = tanh(hidden @ W1^T + enc @ W2^T + b)   # (B, S, H)
    scores = energy . v                             # (B, S)
    out    = softmax(scores, axis=S)[:, None, :]    # (B, 1, S)

Distribution: data-parallel over B — each of the 8 cores handles 4 batch
rows; the small tensors (attn_W, attn_b, v, hidden) are replicated.
No collectives needed; the gather is a host-side concatenation, and the
softmax normalization (a per-row scalar divide) runs on the host too so
the on-chip tail is just exp + DMA.

Precision strategy (error budget is rel<2e-2 on the softmax output):
    The error that k-column k's quantization contributes to the scores is
    weighted by v_k^2, and k-columns are freely permutable (the score sums
    over k; W2 columns, hbias and v are permuted together on the host).
    So: sort k by |v_k| descending. The top 128 columns (k-tile 0, carrying
    ~50% of the v^2 weight) are computed in bf16; the remaining 7 k-tiles
    use fp8 e4m3 with DoubleRow matmuls (2x PE throughput). W2 is
    pre-scaled by 256 before the fp8 cast (unscaled, half its mass sits in
    e4m3's subnormal range; TRN e4m3 saturates at 240 so x256 keeps
    |W2|*256 < 30, far from overflow), and the scale is undone for free
    via the tanh activation's input scale. Per-chunk PE passes:
    8 bf16 + 28 DR + 1 partition-sum, vs 48 for the h-split bf16/fp8
    hybrid, at better accuracy (~1.3e-2 measured vs 1.4e-2).

Scheduling: the PE weight path has only two modes here (bf16 vs fp8
DoubleRow) and each transition flushes the weight prefetch (~260ns).
Chunks are processed in batch-row groups of 4 with phases ordered
    fp8(b0) | bf16(b0) | bf16(b1) | fp8(b1) | fp8(b2) | bf16(b2)
    | fp8(b3) | bf16(b3)
so only 5 transitions occur per kernel (vs 2 per chunk = 32). The
bf16-mode partition-sum matmuls (ones . acc) drain from a rolling queue
inside the bf16 phases, interleaved after each 8-pass kt0 group, so they
never break a DoubleRow run. The warm-up matmuls run in DoubleRow mode on
a memset fp8 tile (DVE memset: GpSimd takes ~6us to boot, DVE starts
immediately) so the kernel both releases the HAM clock gate during the
initial DMA wait and enters phase 1 with the weight path already in DR
mode.

Per-core layout (everything pre-transposed on the host so every DMA is
contiguous):
    encB  (4, 4, 128, 8, 512) bf16  encoder [b][sc][p][ht][s']
    enc8  (4, 4, 128, 4, 2, 512) f8 encoder [b][sc][p][blk][j][s']
                                    (h = blk*256 + j*128 + p)
    w2b   (1, 128, 8, 128)  bf16   W2^T k-tile 0 [p][ht][kcol]
    w28   (7, 128, 4, 2, 128) f8 W2^T*256, k-tiles 1-7 [kt][p][blk][j][kcol]
    hbias (128, 8, 4) f32   hidden @ W1^T + attn_b, tiled (p, kt, b)
    vvf   (128, 8)   f32    v tiled (p, kt)
    ones  (128, 1)   bf16   all-ones (partition-sum stationary)
    out   (4, S)     f32    unnormalized exp(scores)

On-core dataflow (orientation: k on partitions, s on the free axis):
    eT[k, s]   = sum_h w2T[h, k] * encT[h, s]    (main matmul, PSUM f32)
    t[k, s]    = tanh(eT * wscale + hb[k, b])    (ScalarE, per-partition bias)
    acc[k, s]  = sum_kt v[k] * t[k, s]           (VectorE bf16 mul + add chain
                                                  at 2x DVE rate)
    sc[1, s]   = ones . acc                      (matmul partition-sum)
    ex[1, s]   = exp(sc), DMA'd out per chunk (no max subtraction:
                 |scores| <= ||v||_1 ~ 26, exp is safe in f32 and the result
                 is mathematically identical to the max-subtracted softmax;
                 normalization happens on the host)
"""

import numpy as np

B, S, H = 32, 2048, 1024
NCORES = 8
BL = B // NCORES          # batch rows per core
P = 128                   # SBUF partitions
KT = H // P               # 8 k-tiles
HT = H // P               # 8 h-tiles
NBF = 1                   # top-|v| k-tiles computed in bf16
KT0_BF = 4                # h-tiles of k-tile 0 kept in bf16; rest fp8-DR
NB0 = (HT - KT0_BF) // 2  # fp8 DoubleRow blocks for k-tile 0's upper h
NBLK = HT // 2            # fp8 DoubleRow blocks (256 h rows each)
NSC = 4                   # s-chunks per row
SCW = S // NSC            # 512 (one PSUM bank of f32)
W_SCALE = 256.0           # fp8 W2 pre-scale (undone in tanh's input scale)

_CACHE = {}


def _build_nc():
    import concourse.bacc as bacc
    import concourse.mybir as mybir
    import concourse.tile as tile

    dt = mybir.dt
    AFT = mybir.ActivationFunctionType

    nc = bacc.Bacc("TRN2", target_bir_lowering=False, debug=False)

    encB = nc.declare_dram_parameter("encB", [BL, NSC, P, KT0_BF, SCW], dt.bfloat16, isOutput=False)
    enc8 = nc.declare_dram_parameter("enc8", [BL, NSC, P, NBLK, 2, SCW], dt.float8e4, isOutput=False)
    w2b = nc.declare_dram_parameter("w2b", [NBF, P, KT0_BF, P], dt.bfloat16, isOutput=False)
    w280 = nc.declare_dram_parameter("w280", [P, NB0, 2, P], dt.float8e4, isOutput=False)
    w28 = nc.declare_dram_parameter("w28", [KT - NBF, P, NBLK, 2, P], dt.float8e4, isOutput=False)
    hbias = nc.declare_dram_parameter("hbias", [P, KT, BL], dt.float32, isOutput=False)
    vvf = nc.declare_dram_parameter("vvf", [P, KT], dt.float32, isOutput=False)
    ones = nc.declare_dram_parameter("ones", [P, P], dt.bfloat16, isOutput=False)
    warm8 = nc.declare_dram_parameter("warm8", [P, 2, SCW], dt.float8e4, isOutput=False)
    vv8s = nc.declare_dram_parameter("vv8s", [P, NBLK, 2, P], dt.float8e4, isOutput=False)
    vvb0s = nc.declare_dram_parameter("vvb0s", [P, P], dt.bfloat16, isOutput=False)
    out_d = nc.declare_dram_parameter("out", [BL, S], dt.float32, isOutput=True)

    # phase order: fp8-first for b0 (fp8 feeds the PE 12x more work per DMA
    # byte, so the DMA-paced startup idles less) then alternating pairs so
    # adjacent phases share the PE weight-path mode
    FP8_FIRST = {0: True, 1: False, 2: True, 3: True}
    # order in which each b's phases are emitted relative to each other:
    #   b0: fp8, bf16 / b1: bf16, fp8 / b2: fp8, bf16 / b3: fp8, bf16
    # giving the global sequence
    #   f0 | B0 B1 | f1 f2 | B2 | f3 | B3  (5 mode transitions)

    with tile.TileContext(nc) as tc:
        with (
            tc.tile_pool(name="const", bufs=1) as constp,
            tc.tile_pool(name="encb", bufs=9) as encbp,
            tc.tile_pool(name="enc8", bufs=12) as enc8p,
            tc.tile_pool(name="tanh", bufs=3) as tanhp,
            tc.tile_pool(name="th8", bufs=2) as th8p,
            tc.tile_pool(name="accp", bufs=12) as accp,
            tc.tile_pool(name="vtp", bufs=3) as vtp,
            tc.tile_pool(name="soft", bufs=4) as softp,
            tc.tile_pool(name="pe", bufs=6, space="PSUM") as pep,
            tc.tile_pool(name="pv", bufs=1, space="PSUM") as pvp,
            tc.tile_pool(name="pvl", bufs=1, space="PSUM") as pvlp,
        ):
            # PE warm-up in DoubleRow mode: releases the HAM clock gate
            # (1.2 -> 2.4 GHz needs ~3.4us of sustained PE work) while the
            # first weight/enc DMAs are in flight, and leaves the weight
            # path in DR mode for phase 1. The warm-up weights arrive via a
            # tiny DMA on the otherwise-idle scalar ring (~3.5us) — every
            # compute engine has a ~6-8us exec preamble, so a memset on any
            # of them would gate the warm-up later than the DMA does.
            wu8 = constp.tile([P, 2, SCW], dt.float8e4, tag="wu8", name="wu8")
            # warm-up scratch shares the pvl buffer (disjoint lifetime)
            wps = pvlp.tile([P, SCW], dt.float32, tag="pvl", name="wps")
            for _ in range(8):
                nc.tensor.matmul(
                    wps[:], wu8[:, :, 0:P], wu8[:], start=True, stop=True,
                    perf_mode=mybir.MatmulPerfMode.DoubleRow,
                )

            # small constants also on the scalar ring (16 KB)
            hb = constp.tile([P, KT, BL], dt.float32)
            nc.scalar.dma_start(hb[:], hbias.ap())
            vvs = constp.tile([P, KT], dt.float32)
            nc.scalar.dma_start(vvs[:], vvf.ap())
            on1 = constp.tile([P, P], dt.bfloat16)
            nc.scalar.dma_start(on1[:], ones.ap())
            vv8t = constp.tile([P, NBLK, 2, P], dt.float8e4, tag="vv8t", name="vv8t")
            nc.scalar.dma_start(vv8t[:], vv8s.ap())
            vvb0t = constp.tile([P, P], dt.bfloat16, tag="vvb0t", name="vvb0t")
            nc.scalar.dma_start(vvb0t[:], vvb0s.ap())
            # fp8 tanh pair tile for the final chunk's kt7 (partner half
            # stays zero; its vv8s column 8 is zero too)
            th8d = constp.tile([P, 2, SCW], dt.float8e4, tag="th8d", name="th8d")
            nc.gpsimd.memset(th8d[:], 0.0)
            w2bk = constp.tile([P, KT0_BF, P], dt.bfloat16, tag="w2bk", name="w2bk")
            w280k = constp.tile([P, NB0, 2, P], dt.float8e4, tag="w280k", name="w280k")
            w8k = [
                constp.tile([P, NBLK, 2, P], dt.float8e4, tag=f"w8k{kt}",
                            name=f"w8k{kt}")
                for kt in range(KT - NBF)
            ]

            eB_t = {}
            e8_t = {}
            acc_t = {}
            acc_started = set()
            flushq = []

            def alloc_group(b):
                for sc in range(NSC):
                    e8_t[(b, sc)] = enc8p.tile(
                        [P, NBLK, 2, SCW], dt.float8e4, tag="e8t", name="e8t"
                    )
                    eB_t[(b, sc)] = encbp.tile(
                        [P, KT0_BF, SCW], dt.bfloat16, tag="eBt", name="eBt"
                    )
                    acc_t[(b, sc)] = accp.tile([P, SCW], dt.bfloat16, tag="acc", name="acc")

            def dma_group(b):
                """Issue one batch-row's enc DMAs in consumption order."""
                alloc_group(b)
                first = FP8_FIRST[b]
                tiles8 = [(e8_t[(b, sc)], enc8[b][sc]) for sc in range(NSC)]
                tilesB = [(eB_t[(b, sc)], encB[b][sc]) for sc in range(NSC)]
                order = tiles8 + tilesB if first else tilesB + tiles8
                return [nc.sync.dma_start(t[:], src) for t, src in order]

            def chain(b, sc, kt, th):
                """DVE v-dot chain step for chunk (b, sc); appends to the
                flush queue when the chunk's 8th k-tile lands."""
                acc = acc_t[(b, sc)]
                if (b, sc) not in acc_started:
                    acc_started.add((b, sc))
                    nc.vector.tensor_scalar_mul(acc[:], th[:], vvs[:, kt:kt + 1])
                else:
                    vt = vtp.tile([P, SCW], dt.bfloat16, tag="vt", name="vt")
                    nc.vector.tensor_scalar_mul(vt[:], th[:], vvs[:, kt:kt + 1])
                    nc.vector.tensor_add(acc[:], acc[:], vt[:])
                last = kt == (0 if FP8_FIRST[b] else KT - 1)
                if last:
                    flushq.append((b, sc, slot[0]))

            def flush_one():
                """Partition-sum + exp + out-DMA for one finished chunk.
                Only legal inside a bf16 block (the ones-matmul is bf16)."""
                pb, psc, _ = flushq.pop(0)
                # ones is [P,128] with only column 0 nonzero: a free-dim-1
                # stationary disables fast-weight-load (~100ns/pass extra),
                # the wide one doesn't; the sum lands in PSUM row 0
                pv = pvp.tile([P, SCW], dt.float32, name="pv")
                nc.tensor.matmul(pv[:], on1[:], acc_t[(pb, psc)][:], start=True, stop=True)
                ex = softp.tile([1, SCW], dt.float32, tag="ex", name="ex")
                nc.scalar.activation(ex[:], pv[0:1, :], AFT.Exp)
                nc.sync.dma_start(
                    out_d[pb:pb + 1, psc * SCW:(psc + 1) * SCW], ex[:]
                )

            # Both phases run the s-chunk loop INSIDE the weight-block
            # loop with 4 interleaved PSUM accumulation groups (one bank
            # per chunk): each stationary weight block then serves 4
            # consecutive passes, quartering LDWEIGHTS traffic — the
            # partially-exposed weight load is ~25ns/pass otherwise.
            final = {}

            def fp8_final_kt(b, sc, kt, pe):
                # final chunk: the v-dot rides the PE in DoubleRow mode —
                # tanh emits fp8 into pair-packed tiles, and each pair
                # becomes one DR pass against v*W_SCALE weights, so the
                # kernel tail never waits on the DVE chain. kt7 pairs with
                # a zero half (vv8s col 8 is zero as well).
                if kt == 7:
                    t8 = th8d
                    half = 0
                else:
                    if kt % 2 == 1:
                        final["t8"] = th8p.tile([P, 2, SCW], dt.float8e4, tag="t8", name="t8")
                    t8 = final["t8"]
                    half = (kt - 1) % 2
                nc.scalar.activation(
                    t8[:, half, :], pe[:], AFT.Tanh,
                    bias=hb[:, kt, b:b + 1], scale=1.0 / W_SCALE,
                )
                if kt % 2 == 1 and kt != 7:
                    return
                if "pvl" not in final:
                    final["pvl"] = pvlp.tile([P, SCW], dt.float32, tag="pvl", name="pvl")
                pair = (kt - 1) // 2 if kt != 7 else NBLK - 1
                nc.tensor.matmul(
                    final["pvl"][:],
                    vv8t[:, pair, :, :],
                    t8[:],
                    start=(kt == 2),
                    stop=False,
                    perf_mode=mybir.MatmulPerfMode.DoubleRow,
                    skip_group_check=True,
                )

            def fp8_phase(b, chunk_major=False):
                if chunk_major:
                    for sc in range(NSC):
                        for kt in range(NBF, KT):
                            pe = pep.tile([P, SCW], dt.float32, name="pe")
                            for blk in range(NBLK):
                                nc.tensor.matmul(
                                    pe[:],
                                    w8k[kt - NBF][:, blk, :, :],
                                    e8_t[(b, sc)][:, blk, :, :],
                                    start=(blk == 0),
                                    stop=(blk == NBLK - 1),
                                    perf_mode=mybir.MatmulPerfMode.DoubleRow,
                                )
                            th = tanhp.tile([P, SCW], dt.bfloat16, name="th")
                            nc.scalar.activation(
                                th[:], pe[:], AFT.Tanh, bias=hb[:, kt, b:b + 1],
                                scale=1.0 / W_SCALE,
                            )
                            chain(b, sc, kt, th)
                    return
                lastb = b == BL - 1
                pe_t = {}
                for kt in range(NBF, KT):
                    for blk in range(NBLK):
                        for sc in range(NSC):
                            if blk == 0:
                                pe_t[sc] = pep.tile([P, SCW], dt.float32, name="pe")
                            nc.tensor.matmul(
                                pe_t[sc][:],
                                w8k[kt - NBF][:, blk, :, :],
                                e8_t[(b, sc)][:, blk, :, :],
                                start=(blk == 0),
                                stop=(blk == NBLK - 1),
                                perf_mode=mybir.MatmulPerfMode.DoubleRow,
                                skip_group_check=True,
                            )
                            if blk == NBLK - 1:
                                if lastb and sc == NSC - 1:
                                    fp8_final_kt(b, sc, kt, pe_t[sc])
                                    continue
                                th = tanhp.tile([P, SCW], dt.bfloat16, name="th")
                                nc.scalar.activation(
                                    th[:], pe_t[sc][:], AFT.Tanh,
                                    bias=hb[:, kt, b:b + 1],
                                    scale=1.0 / W_SCALE,
                                )
                                chain(b, sc, kt, th)

            slot = [0]

            def bf16_phase(b, chunk_major=False):
                # Mixed k-tile-0 phase: the upper-h contribution runs as a
                # DoubleRow head (adjacent to the neighboring fp8 phase, so
                # it adds no weight-path transition), the bf16 body then
                # CONTINUES the same PSUM accumulation groups with
                # start=False. Both weight halves are pre-scaled by W_SCALE
                # (an exact exponent shift in bf16) so the shared tanh
                # descale stays a single scalar.
                pe_t = {}
                for sc in range(NSC):
                    pe_t[sc] = pep.tile([P, SCW], dt.float32, name="pe")
                    for blk in range(NB0):
                        nc.tensor.matmul(
                            pe_t[sc][:],
                            w280k[:, blk, :, :],
                            e8_t[(b, sc)][:, KT0_BF // 2 + blk, :, :],
                            start=(blk == 0),
                            stop=False,
                            perf_mode=mybir.MatmulPerfMode.DoubleRow,
                            skip_group_check=True,
                        )
                for sc in range(NSC):
                    for ht in range(KT0_BF):
                        nc.tensor.matmul(
                            pe_t[sc][:],
                            w2bk[:, ht, :],
                            eB_t[(b, sc)][:, ht, :],
                            start=False,
                            stop=(ht == KT0_BF - 1),
                            skip_group_check=True,
                        )
                    th = tanhp.tile([P, SCW], dt.bfloat16, name="th")
                    nc.scalar.activation(
                        th[:], pe_t[sc][:], AFT.Tanh, bias=hb[:, 0, b:b + 1],
                        scale=1.0 / W_SCALE,
                    )
                    if b == BL - 1 and sc == NSC - 1:
                        # close the final chunk's PE-side v-dot: kt0's term
                        # via one bf16 pass (v*W_SCALE weights), then exp
                        # with the 1/W_SCALE descale built in
                        nc.tensor.matmul(
                            final["pvl"][:], vvb0t[:], th[:],
                            start=False, stop=True, skip_group_check=True,
                        )
                        ex = softp.tile([1, SCW], dt.float32, tag="ex", name="ex")
                        nc.scalar.activation(
                            ex[:], final["pvl"][0:1, :], AFT.Exp,
                            scale=1.0 / W_SCALE,
                        )
                        nc.sync.dma_start(
                            out_d[b:b + 1, sc * SCW:(sc + 1) * SCW], ex[:]
                        )
                    else:
                        chain(b, sc, 0, th)
                    # drain finished chunks whose DVE chain is >= 2 body
                    # slots old (a fresher one would stall the PE queue on
                    # its partition-sum's acc dependency)
                    slot[0] += 1
                    for _ in range(3):
                        if flushq and slot[0] - flushq[0][2] >= 2:
                            flush_one()

            # ---- phase sequence: f0 | B0 B1 | f1 f2 | B2 | f3 | B3 ----
            # Startup DMAs staged by first-use: the 16 DMA queues run in
            # parallel, so without explicit deps a later 6MB prefetch
            # steals bandwidth from the first passes' weights (measured as
            # an 18.8us PE stall). Each stage waits on the previous one.
            alloc_group(0)
            alloc_group(1)

            def stage(dmas, dep):
                if dep is not None:
                    for d in dmas:
                        tile.add_dep_helper(
                            d.ins, dep.ins,
                            reason="later stage yields startup bandwidth",
                        )
                return dmas[-1]

            # startup DMAs in three stages (deps between stages, parallel
            # queues within a stage): the warm-up weights ride the sync ring
            # (the scalar ring's sequencer is blocked by ACT_TABLE_LOAD);
            # stage B puts the remaining fp8 enc chunks ahead of the
            # remaining fp8 weights — descriptor generation is serial at
            # ~0.65us per DMA, and chunk sc+1's enc gates the PE sooner
            # than k-tile 5's weights do
            # A single dma_start stripes over only a few queues (~5.4us
            # for a 512KB enc chunk), so the first two chunks are split
    
            # into halves for queue parallelism, and the weight doorbells
            # are interleaved in exact consumption order — descriptor
            # generation is serial at ~0.65us each and paces the startup
            e800 = e8_t[(0, 0)]
            e801 = e8_t[(0, 1)]
            sA = stage(
                [
                    nc.sync.dma_start(wu8[:], warm8.ap()),
                    nc.sync.dma_start(e800[:, 0:2, :, :], enc8[0][0][:, 0:2]),
                    nc.sync.dma_start(w8k[0][:], w28[0]),
                    nc.sync.dma_start(e800[:, 2:4, :, :], enc8[0][0][:, 2:4]),
                ],
                None,
            )
            sB = stage(
                [
                    nc.sync.dma_start(w8k[1][:], w28[1]),
                    nc.sync.dma_start(w8k[2][:], w28[2]),
                    nc.sync.dma_start(e801[:, 0:2, :, :], enc8[0][1][:, 0:2]),
                    nc.sync.dma_start(w8k[3][:], w28[3]),
                    nc.sync.dma_start(e801[:, 2:4, :, :], enc8[0][1][:, 2:4]),
                    nc.sync.dma_start(w8k[4][:], w28[4]),
                    nc.sync.dma_start(w8k[5][:], w28[5]),
                    nc.sync.dma_start(w8k[6][:], w28[6]),
                    nc.sync.dma_start(e8_t[(0, 2)][:], enc8[0][2]),
                    nc.sync.dma_start(e8_t[(0, 3)][:], enc8[0][3]),
                ],
                sA,
            )
            sC = stage(
                [nc.sync.dma_start(eB_t[(0, sc)][:], encB[0][sc]) for sc in range(NSC)]
                + [nc.sync.dma_start(w2bk[:], w2b[0]),
                   nc.sync.dma_start(w280k[:], w280.ap())],
                sB,
            )
            stage(
                [nc.sync.dma_start(eB_t[(1, sc)][:], encB[1][sc]) for sc in range(NSC)]
                + [nc.sync.dma_start(e8_t[(1, sc)][:], enc8[1][sc]) for sc in range(NSC)],
                sC,
            )
            fp8_phase(0, chunk_major=True)
            bf16_phase(0)
            bf16_phase(1)
            dma_group(2)
            fp8_phase(1)
            fp8_phase(2)
            dma_group(3)
            bf16_phase(2)
            fp8_phase(3)
            bf16_phase(3)
            while flushq:
                flush_one()

    nc.compile()
    return nc


def _get_nc():
    if "nc" not in _CACHE:
        _CACHE["nc"] = _build_nc()
    return _CACHE["nc"]


def _make_in_maps(hidden, encoder_outputs, attn_W, attn_b, v):
    import concourse.mybir as mybir

    bf16 = mybir.dt.np(mybir.dt.bfloat16)
    f32 = np.float32

    f8 = mybir.dt.np(mybir.dt.float8e4)
    W2T = attn_W[:, H:].T  # (h, k)
    # sort k-columns by |v| descending; permute W2 columns, v and the
    # hidden-term bias together (the score sums over k, so any permutation
    # is exact)
    perm = np.argsort(-np.abs(v))
    W2Ts = np.ascontiguousarray(W2T[:, perm])
    vs = np.ascontiguousarray(np.asarray(v)[perm])
    # k-tile 0 bf16 part (h < KT0_BF*128), scaled by W_SCALE like the fp8
    # parts (exact in bf16) so tanh descales everything with one scalar
    w2b = np.ascontiguousarray(
        (W2Ts[: KT0_BF * P, : NBF * P] * W_SCALE)
        .reshape(KT0_BF, P, NBF, P)
        .transpose(2, 1, 0, 3)
    ).astype(bf16)
    # k-tile 0 fp8 part (h >= KT0_BF*128): [p][blk][j][kcol]
    w280 = np.ascontiguousarray(
        (W2Ts[KT0_BF * P:, : NBF * P] * W_SCALE)
        .reshape(NB0, 2, P, NBF * P)
        .transpose(2, 0, 1, 3)
    ).astype(f8)
    # fp8 k-tiles 1-7, scaled: [kt][p][blk][j][kcol]; h = blk*256 + j*128 + p
    w28 = np.ascontiguousarray(
        (W2Ts[:, NBF * P:] * W_SCALE)
        .reshape(NBLK, 2, P, KT - NBF, P)
        .transpose(3, 2, 0, 1, 4)
    ).astype(f8)
    vvt = np.ascontiguousarray(vs.reshape(KT, P).T).astype(f32)
    ones = np.zeros((P, P), dtype=bf16)
    ones[:, 0] = 1.0
    warm8 = np.zeros((P, 2, SCW), dtype=f8)
    # final-chunk PE v-dot weights, scaled by W_SCALE (unscaled |v|~0.03
    # sits in e4m3's subnormal range). Shaped like the main DR weight
    # blocks ([P, 2, 128], only column 0 nonzero -> result in PSUM row 0)
    # because a narrower stationary fails the dual-fp8 LDWEIGHTS ISA
    # check. Pairs: (kt1,kt2), (kt3,kt4), (kt5,kt6), (kt7, zero pad).
    vsk = vs.reshape(KT, P)
    vv8 = np.zeros((P, NBLK, 2, P), dtype=f8)
    for pair in range(NBLK):
        for j in range(2):
            kt = 1 + 2 * pair + j
            if kt < KT:
                vv8[:, pair, j, 0] = (vsk[kt] * W_SCALE).astype(f8)
    vvb0 = np.zeros((P, P), dtype=bf16)
    vvb0[:, 0] = (vs[:P] * W_SCALE).astype(bf16)
    hid = hidden[0]  # (B, H)
    # hidden-term: (B, H) @ (H, H)^T + b — 8 MFLOP, f32-exact on host
    hterm = (hid @ attn_W[:, :H].T + attn_b)[:, perm].astype(f32)  # (B, H)

    in_maps = []
    for c in range(NCORES):
        sl = slice(c * BL, (c + 1) * BL)
        encs = encoder_outputs[sl]
        # bf16, h < KT0_BF*128 only: [b][sc][p][ht][s']
        encB = np.ascontiguousarray(
            encs[..., : KT0_BF * P]
            .reshape(BL, NSC, SCW, KT0_BF, P)
            .transpose(0, 1, 4, 3, 2)
        ).astype(bf16)
        # fp8 full-h: [b][sc][p][blk][j][s']
        enc8 = np.ascontiguousarray(
            encs.reshape(BL, NSC, SCW, NBLK, 2, P).transpose(0, 1, 5, 3, 4, 2)
        ).astype(f8)
        # hbias[p, kt, b] = hterm[b, kt*128 + p]
        hbias = np.ascontiguousarray(hterm[sl].T.reshape(KT, P, BL).transpose(1, 0, 2))
        in_maps.append(
            {
                "encB": encB,
                "enc8": enc8,
                "w2b": w2b,
                "w280": w280,
                "w28": w28,
                "hbias": hbias,
                "vvf": vvt,
                "ones": ones,
                "warm8": warm8,
                "vv8s": vv8,
                "vvb0s": vvb0,
            }
        )
    return in_maps


def kernel(hidden, encoder_outputs, attn_W, attn_b, v):
    from concourse.bass_utils import run_bass_kernel_spmd

    nc = _get_nc()
    in_maps = _make_in_maps(
        np.asarray(hidden, dtype=np.float32),
        np.asarray(encoder_outputs, dtype=np.float32),
        np.asarray(attn_W, dtype=np.float32),
        np.asarray(attn_b, dtype=np.float32),
        np.asarray(v, dtype=np.float32),
    )
    # A freshly-opened device occasionally fails its first execution with
    # NRT_EXEC_UNIT_UNRECOVERABLE; a retry on the reset device succeeds.
    last_err = None
    for attempt in range(3):
        try:
            res = run_bass_kernel_spmd(nc, in_maps, core_ids=list(range(NCORES)))
            break
        except Exception as e:
            last_err = e
            import time
            time.sleep(2.0)
    else:
        raise last_err
    out = np.concatenate([res.results[c]["out"] for c in range(NCORES)], axis=0)
    # rows hold unnormalized exp(scores); softmax normalization on host
    out = out / out.sum(axis=1, keepdims=True)
    return out[:, None, :].astype(np.float32)
